# revision 14
# baseline (speedup 1.0000x reference)
"""Trainium2 Bass kernel for nn_ContrastiveLoss (SCAN t2i contrastive loss).

Strategy (caption-sharded across 8 cores, per the sharding hint):
  - Each core holds all B=128 images and a 16-caption slice.
  - Gram-matrix identity avoids the (W, D) weighted context per pair:
        P1[w] = sum_r E[r,w] * G[r,w]
        P2[w] = || L_i^T E[:,w] ||^2,   L_i = chol(im_i @ im_i^T)  (host)
    where G = im_i @ cap_c^T, E = exp(9 * leaky_relu(G)/wordnorm).
  - The softmax denominator S = sum_r E cancels inside row_sim, so only
    P1 and P2 are accumulated (on the PE via block-ones matmuls) and
    shipped to the host, which finishes: row_sim = P1/(w1*sqrt(P2)),
    LSE over words, and the tiny (B,B) hinge loss.
  - Ragged-length packing: captions are sorted by length across the
    whole batch; each core gets 8 of the 64 shortest (padded to T1 =
    max short length) and 8 of the 64 longest (padded to T2).  The two
    caption-halves of each core process T1*8- and T2*8-wide word axes
    instead of 400/400 — ~25% less work on every engine.

Engine assignment (single ACT table: {prelu, copy, square, ln, exp} all
live in the natural_log_exp_and_others set -> one ACT_TABLE_LOAD total):
  - ACT: a_t = prelu(G) [PSUM->bf16], rcp = exp(-0.5*ln(nrm+eps)),
         e_t = exp(9*a_t*rcp), sqy = square(Y) [PSUM->bf16]
  - DVE: sq = a_t^2 (bf16 2x), word-norm reduce, an = a_t*rcp,
         prod1 = e_t*G (PSUM)
  - PE : G (bf16), Y = L^T E, block-ones region sums for P1/P2;
         all lhsT operands are 128 columns wide (zero-padded) so every
         LDWEIGHTS takes the fast-weight-load path.

Layout: images padded 128->129, processed as 43 triples of 3 images
(3*36=108 rows, zero-padded to 128 partitions); 2 groups of 22/21
triples accumulate P1/P2 in PSUM (G x3 + Y x2 + P1 + P2 = 7 banks).
"""

import json

import numpy as np

import concourse.bass as bass
import concourse.mybir as mybir
import concourse.tile as tile
from concourse.bass_utils import run_bass_kernel_spmd


def _split_waits(bir_bytes, maxw=1):
    """Walrus in this toolchain accepts only `maxw` sync-waits per
    instruction; hoist extras onto preceding 1-wait Drain no-ops."""
    bir = json.loads(bir_bytes)
    for fn in bir["functions"]:
        for blk in fn["blocks"]:
            out = []
            for inst in blk["instructions"]:
                si = inst.get("sync_info") or {}
                ow = si.get("on_wait") or []
                if len(ow) > maxw:
                    head, tail = ow[:-maxw], ow[-maxw:]
                    for j, w in enumerate(head):
                        out.append({"debug": inst.get("debug"),
                                    "engine": inst["engine"], "ins": [],
                                    "is_reset_sema": False,
                                    "name": f"{inst['name']}-w{j}",
                                    "opcode": "Drain", "outs": [],
                                    "sync_info": {"on_update": [],
                                                  "on_wait": [w]}})
                    si["on_wait"] = tail
                out.append(inst)
            blk["instructions"] = out
    return json.dumps(bir).encode()

F32 = mybir.dt.float32
BF16 = mybir.dt.bfloat16
AF = mybir.ActivationFunctionType
ALU = mybir.AluOpType

LAMBDA_SOFTMAX = 9.0
LAMBDA_LSE = 6.0
MARGIN = 0.2
NRM_EPS = 1e-6

B, R, W, D = 128, 36, 50, 1024
NCORES = 8
CS = B // NCORES            # captions per core
IMG_PAD = 129               # 43 triples of 3 images
NT = IMG_PAD // 3           # 43
TRIP = 3                    # images per triple
PT = TRIP * R               # 108 valid partitions per triple
PTP = 128                   # zero-padded to full width (FWL + tile_pos)
KD = D // 128               # 8 contraction chunks
CH = CS // 2                # 8 captions per half
GROUPS = [(0, 22), (22, 21)]
MG_MAX = max(n for _, n in GROUPS) * TRIP   # 66


def _build_nc(T1, T2):
    wA, wB = CH * T1, CH * T2
    wd = wA + wB
    halves = [(0, wA, T1), (wA, wB, T2)]

    nc = bass.Bass("TRN2", target_bir_lowering=False, debug=False,
                   num_devices=NCORES)

    imT = nc.dram_tensor("imT", [128, NT, KD * PTP], BF16, kind="ExternalInput")
    capT = nc.dram_tensor("capT", [128, KD, wd], BF16, kind="ExternalInput")
    lmat_d = nc.dram_tensor("lmat", [PTP, NT * PTP], BF16, kind="ExternalInput")
    onesb_d = nc.dram_tensor("onesb", [PTP, 2 * MG_MAX], BF16, kind="ExternalInput")
    p1_d = nc.dram_tensor("p1", [IMG_PAD, wd], F32, kind="ExternalOutput")
    p2_d = nc.dram_tensor("p2", [IMG_PAD, wd], F32, kind="ExternalOutput")

    with tile.TileContext(nc) as tc:
        with (
            tc.tile_pool(name="const", bufs=1) as const,
            tc.tile_pool(name="imt", bufs=22) as imtp,
            tc.tile_pool(name="work", bufs=3) as work,
            tc.tile_pool(name="small", bufs=3) as small,
            tc.tile_pool(name="pg", bufs=3, space="PSUM") as pg,
            tc.tile_pool(name="py", bufs=2, space="PSUM") as py,
            tc.tile_pool(name="pacc", bufs=1, space="PSUM") as pacc,
        ):
            # ---- resident constants ----
            cap_sb = const.tile([128, KD, wd], BF16)
            nc.sync.dma_start(out=cap_sb, in_=capT.ap())
            lmat = const.tile([PTP, NT * PTP], BF16)
            nc.sync.dma_start(out=lmat, in_=lmat_d.ap())
            onesb = const.tile([PTP, 2 * MG_MAX], BF16)
            nc.sync.dma_start(out=onesb, in_=onesb_d.ap())
            epsb = const.tile([128, 1], F32)
            nc.vector.memset(epsb, NRM_EPS)

            for t0, ntg in GROUPS:
                mg = ntg * TRIP
                # ---- load this group's image tiles ----
                imt_tiles = []
                for tt in range(ntg):
                    t = t0 + tt
                    imt = imtp.tile([128, KD, PTP], BF16, tag="imt")
                    nc.sync.dma_start(
                        out=imt.rearrange("p k j -> p (k j)"),
                        in_=imT.ap()[:, t, :])
                    imt_tiles.append(imt)

                for w0, hw, tc_len in halves:
                    p1_acc = pacc.tile([MG_MAX, hw], F32, tag="P1",
                                       padded_shape=[MG_MAX, wB])
                    p2_acc = pacc.tile([MG_MAX, hw], F32, tag="P2",
                                       padded_shape=[MG_MAX, wB])
                    mm_flags = lambda tt: dict(
                        start=(tt == 0), stop=(tt == ntg - 1),
                        skip_group_check=True)

                    def lhs_ones(tt):
                        return onesb[:, MG_MAX - TRIP * tt:
                                     MG_MAX - TRIP * tt + mg]

                    # three-stage software pipeline over triples: every
                    # engine's stream only contains ops whose inputs were
                    # produced >= 1 step earlier, so nothing head-of-line
                    # blocks.  Step tt:
                    #   PE : G(tt)x8 | Y(tt-1) P1(tt-1) | P2(tt-2)
                    #   ACT: prelu(tt) | ln,exprcp(tt-1) | sqy(tt-2)
                    #        | exp9(tt-1)
                    #   DVE: sq,reduce(tt) | an,prod1(tt-1)
                    st = {}
                    for step in range(ntg + 2):
                        tt, t1_, t2_ = step, step - 1, step - 2
                        # ---- PE: G burst for tt ----
                        if tt < ntg:
                            imt = imt_tiles[tt]
                            gps = pg.tile([PTP, hw], F32, tag="G",
                                          padded_shape=[PTP, wB])
                            for k in range(KD):
                                nc.tensor.matmul(
                                    gps, lhsT=imt[:, k, :],
                                    rhs=cap_sb[:, k, w0:w0 + hw],
                                    start=(k == 0), stop=(k == KD - 1))
                            st[tt] = {"gps": gps}
                            # ACT/DVE stage 0 for tt
                            s = st[tt]
                            a_t = work.tile([PTP, hw], BF16, tag="a",
                                            padded_shape=[PTP, wB])
                            nc.scalar.activation(out=a_t, in_=s["gps"],
                                                 func=AF.Prelu, alpha=0.1)
                            s["a"] = a_t
                            sq = work.tile([PTP, hw], BF16, tag="s",
                                           padded_shape=[PTP, wB])
                            nc.vector.tensor_tensor(out=sq, in0=a_t,
                                                    in1=a_t, op=ALU.mult)
                            nrm = small.tile([PTP, CH], F32, tag="n")
                            nc.vector.tensor_reduce(
                                out=nrm,
                                in_=sq.rearrange("p (c w) -> p c w", w=tc_len),
                                axis=mybir.AxisListType.X, op=ALU.add)
                            s["nrm"] = nrm
                        # ---- stage 1 for t1_: norm -> E -> prod1 ----
                        if 0 <= t1_ < ntg:
                            s = st[t1_]
                            lnn = small.tile([PTP, CH], F32, tag="l")
                            nc.scalar.activation(out=lnn, in_=s["nrm"],
                                                 func=AF.Ln, bias=epsb[:PTP, :])
                            rcp = small.tile([PTP, CH], BF16, tag="r")
                            nc.scalar.activation(out=rcp, in_=lnn, func=AF.Exp,
                                                 scale=-0.5)
                            an = work.tile([PTP, hw], BF16, tag="an",
                                           padded_shape=[PTP, wB])
                            nc.vector.tensor_tensor(
                                out=an.rearrange("p (c w) -> p c w", w=tc_len),
                                in0=s["a"].rearrange("p (c w) -> p c w",
                                                     w=tc_len),
                                in1=rcp[:, :, None]
                                .to_broadcast([PTP, CH, tc_len]),
                                op=ALU.mult)
                            e_t = work.tile([PTP, hw], BF16, tag="e",
                                            padded_shape=[PTP, wB])
                            nc.scalar.activation(out=e_t, in_=an, func=AF.Exp,
                                                 scale=LAMBDA_SOFTMAX)
                            s["e"] = e_t
                            prod1 = work.tile([PTP, hw], BF16, tag="p",
                                              padded_shape=[PTP, wB])
                            nc.vector.tensor_tensor(out=prod1, in0=e_t,
                                                    in1=s["gps"], op=ALU.mult)
                            s["p1"] = prod1
                        # ---- stage 2 for t2_: square(Y) ----
                        if 0 <= t2_ < ntg:
                            s = st[t2_]
                            sqy = work.tile([PTP, hw], BF16, tag="q",
                                            padded_shape=[PTP, wB])
                            nc.scalar.square(out=sqy, in_=s["yps"])
                            s["sqy"] = sqy
                        # ---- PE: Y/P1 for t1_, P2 for t2_ ----
                        if 0 <= t1_ < ntg:
                            s = st[t1_]
                            yps = py.tile([PTP, hw], F32, tag="Y",
                                          padded_shape=[PTP, wB])
                            nc.tensor.matmul(
                                yps,
                                lhsT=lmat[:, (t0 + t1_) * PTP:
                                          (t0 + t1_ + 1) * PTP],
                                rhs=s["e"], start=True, stop=True)
                            s["yps"] = yps
                            nc.tensor.matmul(p1_acc[:mg], lhsT=lhs_ones(t1_),
                                             rhs=s["p1"], **mm_flags(t1_))
                        if 0 <= t2_ < ntg:
                            nc.tensor.matmul(p2_acc[:mg], lhsT=lhs_ones(t2_),
                                             rhs=st[t2_]["sqy"],
                                             **mm_flags(t2_))
                            st.pop(t2_, None)

                    p1_sb = work.tile([MG_MAX, hw], F32, tag="o1",
                                      padded_shape=[MG_MAX, wB])
                    nc.vector.tensor_copy(out=p1_sb[:mg], in_=p1_acc[:mg])
                    nc.sync.dma_start(
                        out=p1_d.ap()[t0 * TRIP:t0 * TRIP + mg, w0:w0 + hw],
                        in_=p1_sb[:mg])
                    p2_sb = work.tile([MG_MAX, hw], F32, tag="o2",
                                      padded_shape=[MG_MAX, wB])
                    nc.vector.tensor_copy(out=p2_sb[:mg], in_=p2_acc[:mg])
                    nc.sync.dma_start(
                        out=p2_d.ap()[t0 * TRIP:t0 * TRIP + mg, w0:w0 + hw],
                        in_=p2_sb[:mg])

    _orig = nc.to_json_bytes
    nc.to_json_bytes = lambda *a, **k: _split_waits(_orig(*a, **k))
    return nc


_NC_CACHE = {}
# test-harness hooks (harmless defaults for grading)
TRACE = False
LAST_RESULTS = None


def _bf16(x):
    import ml_dtypes
    return np.ascontiguousarray(x, np.float32).astype(ml_dtypes.bfloat16)


def _host_prep(im, s, s_l):
    im = np.ascontiguousarray(np.asarray(im, np.float32))
    s = np.asarray(s, np.float32)
    L = np.asarray(s_l).astype(np.int64)
    mask = (np.arange(W)[None, :] < L[:, None]).astype(np.float32)
    cap = np.ascontiguousarray(s * mask[:, :, None])

    # caption assignment: sort by length; each core takes 8 short + 8 long
    order = np.argsort(L, kind="stable")
    shorts, longs = order[:B // 2], order[B // 2:]
    T1 = int(L[shorts].max())
    T2 = int(L[longs].max())
    glob = np.stack([np.concatenate([shorts[c::NCORES], longs[c::NCORES]])
                     for c in range(NCORES)])           # (ncores, 16)
    wA, wB = CH * T1, CH * T2
    wd = wA + wB

    imf = np.concatenate(
        [im.reshape(B * R, D), np.zeros(((IMG_PAD - B) * R, D), np.float32)], 0)
    # [128, NT, KD*PTP]: per-triple contiguous, region rows padded 108->128
    imq = imf.T.reshape(KD, 128, NT, PT)
    imp = np.zeros((KD, 128, NT, PTP), np.float32)
    imp[:, :, :, :PT] = imq
    imT = _bf16(np.ascontiguousarray(
        imp.transpose(1, 2, 0, 3).reshape(128, NT, KD * PTP)))

    # Cholesky factors of per-image Gram matrices, block-diag per triple.
    gram = np.einsum('pad,pbd->pab', imf.reshape(IMG_PAD, R, D),
                     imf.reshape(IMG_PAD, R, D), optimize=True)
    gram += 1e-3 * np.eye(R, dtype=np.float32)
    lch = np.linalg.cholesky(gram.astype(np.float64)).astype(np.float32)
    lmat = np.zeros((PTP, NT * PTP), np.float32)
    for t in range(NT):
        for j in range(TRIP):
            i = t * TRIP + j
            lmat[j * R:(j + 1) * R, t * PTP + j * R:t * PTP + (j + 1) * R] = \
                lch[i]
    lmat = _bf16(lmat)

    onesb = np.zeros((PTP, 2 * MG_MAX), np.float32)
    for j in range(TRIP):
        onesb[j * R:(j + 1) * R, MG_MAX + j] = 1.0
    onesb = _bf16(onesb)

    # packed caption tensors + epilogue masks/norms in slot order
    in_maps = []
    mask_pk = np.zeros((NCORES, CS, max(T1, T2)), np.float32)
    w1_pk = np.zeros((NCORES, CS, max(T1, T2)), np.float32)
    for c in range(NCORES):
        capf = np.zeros((wd, D), np.float32)
        for slot in range(CS):
            g = glob[c, slot]
            tl = T1 if slot < CH else T2
            o = slot * T1 if slot < CH else wA + (slot - CH) * T2
            capf[o:o + tl] = cap[g, :tl]
            mask_pk[c, slot, :tl] = mask[g, :tl]
            w1_pk[c, slot, :tl] = np.sqrt((cap[g, :tl] ** 2).sum(-1))
        capT = _bf16(np.ascontiguousarray(
            capf.T.reshape(KD, 128, wd).transpose(1, 0, 2)))
        in_maps.append({"imT": imT, "capT": capT, "lmat": lmat,
                        "onesb": onesb})
    return in_maps, (T1, T2, wA, wd, glob, mask_pk, w1_pk)


def kernel(im, im_l, s, s_l):
    global LAST_RESULTS
    in_maps, meta = _host_prep(im, s, s_l)
    T1, T2, wA, wd, glob, mask_pk, w1_pk = meta
    if (T1, T2) not in _NC_CACHE:
        _NC_CACHE[(T1, T2)] = _build_nc(T1, T2)
    res = run_bass_kernel_spmd(_NC_CACHE[(T1, T2)], in_maps,
                               core_ids=list(range(NCORES)), trace=TRACE)
    LAST_RESULTS = res
    # host epilogue: row_sim -> masked LSE over words -> scores
    scores = np.zeros((B, B), np.float32)
    for c, r in enumerate(res.results):
        p1 = np.asarray(r["p1"][:B], np.float32)
        p2 = np.asarray(r["p2"][:B], np.float32)
        for slot in range(CS):
            tl = T1 if slot < CH else T2
            o = slot * T1 if slot < CH else wA + (slot - CH) * T2
            q1, q2 = p1[:, o:o + tl], p2[:, o:o + tl]
            w1 = w1_pk[c, slot, :tl][None]
            den = np.maximum(w1 * np.sqrt(np.maximum(q2, 1e-30)), 1e-4)
            xx = np.exp(q1 / den * LAMBDA_LSE) * mask_pk[c, slot, :tl][None]
            scores[:, glob[c, slot]] = np.log(xx.sum(-1)) / LAMBDA_LSE

    diag = np.diagonal(scores)[:, None]
    cost_s = np.maximum(MARGIN + scores - diag, 0.0)
    cost_im = np.maximum(MARGIN + scores - diag.T, 0.0)
    np.fill_diagonal(cost_s, 0.0)
    np.fill_diagonal(cost_im, 0.0)
    loss = np.sum(np.max(cost_s, axis=1)) + np.sum(np.max(cost_im, axis=0))
    return np.array(loss, np.float32)


# revision 15
# speedup vs baseline: 1.4432x; 1.4432x over previous
"""Trainium2 Bass kernel for nn_ContrastiveLoss (SCAN t2i contrastive loss).

Strategy (caption-sharded across 8 cores, per the sharding hint):
  - Each core holds all B=128 images and a 16-caption slice.
  - Gram-matrix identity avoids the (W, D) weighted context per pair:
        P1[w] = sum_r E[r,w] * G[r,w]
        P2[w] = || L_i^T E[:,w] ||^2,   L_i = chol(im_i @ im_i^T)  (host)
    where G = im_i @ cap_c^T, E = exp(9 * leaky_relu(G)/wordnorm).
  - The softmax denominator S = sum_r E cancels inside row_sim, so only
    P1 and P2 are accumulated (on the PE via block-ones matmuls) and
    shipped to the host, which finishes: row_sim = P1/(w1*sqrt(P2)),
    LSE over words, and the tiny (B,B) hinge loss.
  - Ragged-length packing: captions are sorted by length across the
    whole batch; each core gets 8 of the 64 shortest (padded to T1 =
    max short length) and 8 of the 64 longest (padded to T2).  The two
    caption-halves of each core process T1*8- and T2*8-wide word axes
    instead of 400/400 — ~25% less work on every engine.

Engine assignment (single ACT table: {prelu, copy, square, ln, exp} all
live in the natural_log_exp_and_others set -> one ACT_TABLE_LOAD total):
  - ACT: a_t = prelu(G) [PSUM->bf16], rcp = exp(-0.5*ln(nrm+eps)),
         e_t = exp(9*a_t*rcp), sqy = square(Y) [PSUM->bf16]
  - DVE: sq = a_t^2 (bf16 2x), word-norm reduce, an = a_t*rcp,
         prod1 = e_t*G (PSUM)
  - PE : G (bf16), Y = L^T E, block-ones region sums for P1/P2;
         all lhsT operands are 128 columns wide (zero-padded) so every
         LDWEIGHTS takes the fast-weight-load path.

Layout: images padded 128->129, processed as 43 triples of 3 images
(3*36=108 rows, zero-padded to 128 partitions); 2 groups of 22/21
triples accumulate P1/P2 in PSUM (G x3 + Y x2 + P1 + P2 = 7 banks).
"""

import json

import numpy as np

import concourse.bass as bass
import concourse.mybir as mybir
import concourse.tile as tile
from concourse.bass_utils import run_bass_kernel_spmd


def _split_waits(bir_bytes, maxw=1):
    """Walrus in this toolchain accepts only `maxw` sync-waits per
    instruction; hoist extras onto preceding 1-wait Drain no-ops."""
    bir = json.loads(bir_bytes)
    for fn in bir["functions"]:
        for blk in fn["blocks"]:
            out = []
            for inst in blk["instructions"]:
                si = inst.get("sync_info") or {}
                ow = si.get("on_wait") or []
                if len(ow) > maxw:
                    head, tail = ow[:-maxw], ow[-maxw:]
                    for j, w in enumerate(head):
                        out.append({"debug": inst.get("debug"),
                                    "engine": inst["engine"], "ins": [],
                                    "is_reset_sema": False,
                                    "name": f"{inst['name']}-w{j}",
                                    "opcode": "Drain", "outs": [],
                                    "sync_info": {"on_update": [],
                                                  "on_wait": [w]}})
                    si["on_wait"] = tail
                out.append(inst)
            blk["instructions"] = out
    return json.dumps(bir).encode()

F32 = mybir.dt.float32
BF16 = mybir.dt.bfloat16
AF = mybir.ActivationFunctionType
ALU = mybir.AluOpType

LAMBDA_SOFTMAX = 9.0
LAMBDA_LSE = 6.0
MARGIN = 0.2
NRM_EPS = 1e-6

B, R, W, D = 128, 36, 50, 1024
NCORES = 8
CS = B // NCORES            # captions per core
IMG_PAD = 129               # 43 triples of 3 images
NT = IMG_PAD // 3           # 43
TRIP = 3                    # images per triple
PT = TRIP * R               # 108 valid partitions per triple
PTP = 128                   # zero-padded to full width (FWL + tile_pos)
KD = D // 128               # 8 contraction chunks
CH = CS // 2                # 8 captions per half
GROUPS = [(0, 22), (22, 21)]
MG_MAX = max(n for _, n in GROUPS) * TRIP   # 66


def _build_nc(T1, T2):
    wA, wB = CH * T1, CH * T2
    wd = wA + wB
    halves = [(0, wA, T1), (wA, wB, T2)]

    nc = bass.Bass("TRN2", target_bir_lowering=False, debug=False,
                   num_devices=NCORES)

    imT = nc.dram_tensor("imT", [128, NT, KD * PTP], BF16, kind="ExternalInput")
    capT = nc.dram_tensor("capT", [128, KD, wd], BF16, kind="ExternalInput")
    lmat_d = nc.dram_tensor("lmat", [PTP, NT * PTP], BF16, kind="ExternalInput")
    onesb_d = nc.dram_tensor("onesb", [PTP, 2 * MG_MAX], BF16, kind="ExternalInput")
    p1_d = nc.dram_tensor("p1", [IMG_PAD, wd], F32, kind="ExternalOutput")
    p2_d = nc.dram_tensor("p2", [IMG_PAD, wd], F32, kind="ExternalOutput")

    with tile.TileContext(nc) as tc:
        with (
            tc.tile_pool(name="const", bufs=1) as const,
            tc.tile_pool(name="imt", bufs=22) as imtp,
            tc.tile_pool(name="work", bufs=3) as work,
            tc.tile_pool(name="small", bufs=3) as small,
            tc.tile_pool(name="pg", bufs=4, space="PSUM") as pg,
            tc.tile_pool(name="py", bufs=2, space="PSUM") as py,
            tc.tile_pool(name="pacc", bufs=1, space="PSUM") as pacc,
        ):
            # ---- resident constants ----
            cap_sb = const.tile([128, KD, wd], BF16)
            nc.sync.dma_start(out=cap_sb, in_=capT.ap())
            lmat = const.tile([PTP, NT * PTP], BF16)
            nc.sync.dma_start(out=lmat, in_=lmat_d.ap())
            onesb = const.tile([PTP, 2 * MG_MAX], BF16)
            nc.sync.dma_start(out=onesb, in_=onesb_d.ap())
            epsb = const.tile([128, 1], F32)
            nc.vector.memset(epsb, NRM_EPS)

            for t0, ntg in GROUPS:
                mg = ntg * TRIP
                # ---- load this group's image tiles ----
                imt_tiles = []
                for tt in range(ntg):
                    t = t0 + tt
                    imt = imtp.tile([128, KD, PTP], BF16, tag="imt")
                    nc.sync.dma_start(
                        out=imt.rearrange("p k j -> p (k j)"),
                        in_=imT.ap()[:, t, :])
                    imt_tiles.append(imt)

                for w0, hw, tc_len in halves:
                    p1_acc = pacc.tile([MG_MAX, hw], F32, tag="P1",
                                       padded_shape=[MG_MAX, wB])
                    p2_acc = pacc.tile([MG_MAX, hw], F32, tag="P2",
                                       padded_shape=[MG_MAX, wB])
                    mm_flags = lambda tt: dict(
                        start=(tt == 0), stop=(tt == ntg - 1),
                        skip_group_check=True)

                    def lhs_ones(tt):
                        return onesb[:, MG_MAX - TRIP * tt:
                                     MG_MAX - TRIP * tt + mg]

                    # three-stage software pipeline over triples: every
                    # engine's stream only contains ops whose inputs were
                    # produced >= 1 step earlier, so nothing head-of-line
                    # blocks.  Step tt:
                    #   PE : G(tt)x8 | Y(tt-1) P1(tt-1) | P2(tt-2)
                    #   ACT: prelu(tt) | ln,exprcp(tt-1) | sqy(tt-2)
                    #        | exp9(tt-1)
                    #   DVE: sq,reduce(tt) | an,prod1(tt-1)
                    st = {}
                    for step in range(ntg + 2):
                        tt, t1_, t2_ = step, step - 1, step - 2
                        # ---- PE: G burst for tt ----
                        if tt < ntg:
                            imt = imt_tiles[tt]
                            gps = pg.tile([PTP, hw], F32, tag="G",
                                          padded_shape=[PTP, wB])
                            for k in range(KD):
                                nc.tensor.matmul(
                                    gps, lhsT=imt[:, k, :],
                                    rhs=cap_sb[:, k, w0:w0 + hw],
                                    start=(k == 0), stop=(k == KD - 1))
                            st[tt] = {"gps": gps}
                            # ACT/DVE stage 0 for tt
                            s = st[tt]
                            a_t = work.tile([PTP, hw], BF16, tag="a",
                                            padded_shape=[PTP, wB])
                            nc.scalar.activation(out=a_t, in_=s["gps"],
                                                 func=AF.Prelu, alpha=0.1)
                            s["a"] = a_t
                            sq = work.tile([PTP, hw], BF16, tag="s",
                                           padded_shape=[PTP, wB])
                            nc.vector.tensor_tensor(out=sq, in0=a_t,
                                                    in1=a_t, op=ALU.mult)
                            nrm = small.tile([PTP, CH], F32, tag="n")
                            nc.vector.tensor_reduce(
                                out=nrm,
                                in_=sq.rearrange("p (c w) -> p c w", w=tc_len),
                                axis=mybir.AxisListType.X, op=ALU.add)
                            s["nrm"] = nrm
                        # ---- stage 1 for t1_: norm -> E -> prod1 ----
                        if 0 <= t1_ < ntg:
                            s = st[t1_]
                            lnn = small.tile([PTP, CH], F32, tag="l")
                            nc.scalar.activation(out=lnn, in_=s["nrm"],
                                                 func=AF.Ln, bias=epsb[:PTP, :])
                            rcp = small.tile([PTP, CH], BF16, tag="r")
                            nc.scalar.activation(out=rcp, in_=lnn, func=AF.Exp,
                                                 scale=-0.5)
                            an = work.tile([PTP, hw], BF16, tag="an",
                                           padded_shape=[PTP, wB])
                            nc.vector.tensor_tensor(
                                out=an.rearrange("p (c w) -> p c w", w=tc_len),
                                in0=s["a"].rearrange("p (c w) -> p c w",
                                                     w=tc_len),
                                in1=rcp[:, :, None]
                                .to_broadcast([PTP, CH, tc_len]),
                                op=ALU.mult)
                            e_t = work.tile([PTP, hw], BF16, tag="e",
                                            padded_shape=[PTP, wB])
                            nc.scalar.activation(out=e_t, in_=an, func=AF.Exp,
                                                 scale=LAMBDA_SOFTMAX)
                            s["e"] = e_t
                            prod1 = work.tile([PTP, hw], BF16, tag="p",
                                              padded_shape=[PTP, wB])
                            nc.vector.tensor_tensor(out=prod1, in0=e_t,
                                                    in1=s["gps"], op=ALU.mult)
                            s["p1"] = prod1
                        # ---- stage 2 for t2_: square(Y) ----
                        if 0 <= t2_ < ntg:
                            s = st[t2_]
                            sqy = work.tile([PTP, hw], BF16, tag="q",
                                            padded_shape=[PTP, wB])
                            nc.scalar.square(out=sqy, in_=s["yps"])
                            s["sqy"] = sqy
                        # ---- PE: Y/P1 for t1_, P2 for t2_ ----
                        if 0 <= t1_ < ntg:
                            s = st[t1_]
                            yps = py.tile([PTP, hw], F32, tag="Y",
                                          padded_shape=[PTP, wB])
                            nc.tensor.matmul(
                                yps,
                                lhsT=lmat[:, (t0 + t1_) * PTP:
                                          (t0 + t1_ + 1) * PTP],
                                rhs=s["e"], start=True, stop=True)
                            s["yps"] = yps
                            nc.tensor.matmul(p1_acc[:mg], lhsT=lhs_ones(t1_),
                                             rhs=s["p1"], **mm_flags(t1_))
                        if 0 <= t2_ < ntg:
                            nc.tensor.matmul(p2_acc[:mg], lhsT=lhs_ones(t2_),
                                             rhs=st[t2_]["sqy"],
                                             **mm_flags(t2_))
                            st.pop(t2_, None)

                    p1_sb = work.tile([MG_MAX, hw], F32, tag="o1",
                                      padded_shape=[MG_MAX, wB])
                    nc.vector.tensor_copy(out=p1_sb[:mg], in_=p1_acc[:mg])
                    nc.sync.dma_start(
                        out=p1_d.ap()[t0 * TRIP:t0 * TRIP + mg, w0:w0 + hw],
                        in_=p1_sb[:mg])
                    p2_sb = work.tile([MG_MAX, hw], F32, tag="o2",
                                      padded_shape=[MG_MAX, wB])
                    nc.vector.tensor_copy(out=p2_sb[:mg], in_=p2_acc[:mg])
                    nc.sync.dma_start(
                        out=p2_d.ap()[t0 * TRIP:t0 * TRIP + mg, w0:w0 + hw],
                        in_=p2_sb[:mg])

    _orig = nc.to_json_bytes
    nc.to_json_bytes = lambda *a, **k: _split_waits(_orig(*a, **k))
    return nc


_NC_CACHE = {}
# test-harness hooks (harmless defaults for grading)
TRACE = False
LAST_RESULTS = None


def _bf16(x):
    import ml_dtypes
    return np.ascontiguousarray(x, np.float32).astype(ml_dtypes.bfloat16)


def _host_prep(im, s, s_l):
    im = np.ascontiguousarray(np.asarray(im, np.float32))
    s = np.asarray(s, np.float32)
    L = np.asarray(s_l).astype(np.int64)
    mask = (np.arange(W)[None, :] < L[:, None]).astype(np.float32)
    cap = np.ascontiguousarray(s * mask[:, :, None])

    # caption assignment: sort by length; each core takes 8 short + 8 long
    order = np.argsort(L, kind="stable")
    shorts, longs = order[:B // 2], order[B // 2:]
    T1 = int(L[shorts].max())
    T2 = int(L[longs].max())
    glob = np.stack([np.concatenate([shorts[c::NCORES], longs[c::NCORES]])
                     for c in range(NCORES)])           # (ncores, 16)
    wA, wB = CH * T1, CH * T2
    wd = wA + wB

    imf = np.concatenate(
        [im.reshape(B * R, D), np.zeros(((IMG_PAD - B) * R, D), np.float32)], 0)
    # [128, NT, KD*PTP]: per-triple contiguous, region rows padded 108->128
    imq = imf.T.reshape(KD, 128, NT, PT)
    imp = np.zeros((KD, 128, NT, PTP), np.float32)
    imp[:, :, :, :PT] = imq
    imT = _bf16(np.ascontiguousarray(
        imp.transpose(1, 2, 0, 3).reshape(128, NT, KD * PTP)))

    # Cholesky factors of per-image Gram matrices, block-diag per triple.
    gram = np.einsum('pad,pbd->pab', imf.reshape(IMG_PAD, R, D),
                     imf.reshape(IMG_PAD, R, D), optimize=True)
    gram += 1e-3 * np.eye(R, dtype=np.float32)
    lch = np.linalg.cholesky(gram.astype(np.float64)).astype(np.float32)
    lmat = np.zeros((PTP, NT * PTP), np.float32)
    for t in range(NT):
        for j in range(TRIP):
            i = t * TRIP + j
            lmat[j * R:(j + 1) * R, t * PTP + j * R:t * PTP + (j + 1) * R] = \
                lch[i]
    lmat = _bf16(lmat)

    onesb = np.zeros((PTP, 2 * MG_MAX), np.float32)
    for j in range(TRIP):
        onesb[j * R:(j + 1) * R, MG_MAX + j] = 1.0
    onesb = _bf16(onesb)

    # packed caption tensors + epilogue masks/norms in slot order
    in_maps = []
    mask_pk = np.zeros((NCORES, CS, max(T1, T2)), np.float32)
    w1_pk = np.zeros((NCORES, CS, max(T1, T2)), np.float32)
    for c in range(NCORES):
        capf = np.zeros((wd, D), np.float32)
        for slot in range(CS):
            g = glob[c, slot]
            tl = T1 if slot < CH else T2
            o = slot * T1 if slot < CH else wA + (slot - CH) * T2
            capf[o:o + tl] = cap[g, :tl]
            mask_pk[c, slot, :tl] = mask[g, :tl]
            w1_pk[c, slot, :tl] = np.sqrt((cap[g, :tl] ** 2).sum(-1))
        capT = _bf16(np.ascontiguousarray(
            capf.T.reshape(KD, 128, wd).transpose(1, 0, 2)))
        in_maps.append({"imT": imT, "capT": capT, "lmat": lmat,
                        "onesb": onesb})
    return in_maps, (T1, T2, wA, wd, glob, mask_pk, w1_pk)


def kernel(im, im_l, s, s_l):
    global LAST_RESULTS
    in_maps, meta = _host_prep(im, s, s_l)
    T1, T2, wA, wd, glob, mask_pk, w1_pk = meta
    if (T1, T2) not in _NC_CACHE:
        _NC_CACHE[(T1, T2)] = _build_nc(T1, T2)
    res = run_bass_kernel_spmd(_NC_CACHE[(T1, T2)], in_maps,
                               core_ids=list(range(NCORES)), trace=TRACE)
    LAST_RESULTS = res
    # host epilogue: row_sim -> masked LSE over words -> scores
    scores = np.zeros((B, B), np.float32)
    for c, r in enumerate(res.results):
        p1 = np.asarray(r["p1"][:B], np.float32)
        p2 = np.asarray(r["p2"][:B], np.float32)
        for slot in range(CS):
            tl = T1 if slot < CH else T2
            o = slot * T1 if slot < CH else wA + (slot - CH) * T2
            q1, q2 = p1[:, o:o + tl], p2[:, o:o + tl]
            w1 = w1_pk[c, slot, :tl][None]
            den = np.maximum(w1 * np.sqrt(np.maximum(q2, 1e-30)), 1e-4)
            xx = np.exp(q1 / den * LAMBDA_LSE) * mask_pk[c, slot, :tl][None]
            scores[:, glob[c, slot]] = np.log(xx.sum(-1)) / LAMBDA_LSE

    diag = np.diagonal(scores)[:, None]
    cost_s = np.maximum(MARGIN + scores - diag, 0.0)
    cost_im = np.maximum(MARGIN + scores - diag.T, 0.0)
    np.fill_diagonal(cost_s, 0.0)
    np.fill_diagonal(cost_im, 0.0)
    loss = np.sum(np.max(cost_s, axis=1)) + np.sum(np.max(cost_im, axis=0))
    return np.array(loss, np.float32)
